# revision 3
# baseline (speedup 1.0000x reference)
"""Trainium2 Bass kernel v3 for CoreAttention (GQA, additive mask, softmax).

Per batch b, head h, kv-group g = h // 16:
    scores = (Q[b,h] @ K[b,g].T) / sqrt(128) + mask[b,0]
    attn   = softmax(scores, axis=-1)
    out    = attn @ V[b,g]
    context[q, b, h*128:(h+1)*128] = out[q]

Sharding: 64 (b,h) pairs -> 8 cores x 8 heads; all heads on a core share
one kv head.  All I/O fp16 (inputs are consumed in fp16 by the PE anyway;
fp16 I/O halves both HBM and host-link traffic).

Per-core flow (transposed-score layout, kv on partitions):
    expMT = exp(mask^T)        fp16 tiles, built once per pass:
                               DMA mask (q-major) -> PE fp16 transpose
                               -> ACT exp (PSUM->SBUF, strided out)
    per head:
      Q^T  : DMA q tiles -> PE fp16 transpose -> DVE copy -> QT [d, q]
      S^T  = KT_c^T @ QT         (PE, 2x512-col matmuls per 1024-q half)
      P    = exp(S^T*scale - 4)  (ACT, one [128,1024] instr per chunk)
      P   *= expMT               (DVE, fp16 2x mode)
      av   = P_chunk^T @ [V | 1] (PE, PSUM-accumulated over 16 chunks;
                                  col 128 = softmax denominator)
      out  = av[:, :128] * recip(av[:, 128])  (DVE), DMA out fp16
"""

import math
import sys

import numpy as np

try:
    import concourse.bass as bass
except ModuleNotFoundError:  # fresh grading dir: repo lives at /opt
    sys.path.insert(0, "/opt/trn_rl_repo")
    import concourse.bass as bass

import concourse.mybir as mybir
import concourse.tile as tile
from concourse import bacc

F32 = mybir.dt.float32
F16 = mybir.dt.float16
EXPF = mybir.ActivationFunctionType.Exp

# Problem constants (nn_CoreAttention_35493609734503)
B, H, G = 2, 32, 2
QLEN, KVLEN, D = 2048, 2048, 128
N_CORES = 8
HEADS_PER_CORE = (B * H) // N_CORES  # 8
SCALE = 1.0 / math.sqrt(D)  # /(sqrt(d)*coeff) * coeff
EXP_BIAS = -4.0  # exp(x-4) keeps fp16 exp values small; cancels in softmax


def build_program(n_heads=HEADS_PER_CORE, qlen=QLEN, kvlen=KVLEN, repeat=1,
                  hw_loop=False):
    nc = bacc.Bacc("TRN2", target_bir_lowering=False)
    d = D
    q_dram = nc.dram_tensor("q", [n_heads, qlen, d], F16, kind="ExternalInput").ap()
    kt_dram = nc.dram_tensor("kt", [d, kvlen], F16, kind="ExternalInput").ap()
    v1_dram = nc.dram_tensor(
        "v1", [128, kvlen // 128, d + 1], F16, kind="ExternalInput"
    ).ap()
    m_dram = nc.dram_tensor("mask", [qlen, kvlen], F16, kind="ExternalInput").ap()
    o_dram = nc.dram_tensor("out", [n_heads, qlen, d], F16, kind="ExternalOutput").ap()

    NKV = kvlen // 128  # kv chunks (kv on partitions in S^T)
    NQT = qlen // 128  # q tiles
    QHS = min(1024, qlen)  # q half size (PSUM-bounded)
    NQH = qlen // QHS
    QSUB = QHS // 128

    from concourse.masks import make_identity

    with tile.TileContext(nc) as tc:
        with (
            tc.tile_pool(name="const", bufs=1) as constp,
            tc.tile_pool(name="ktp", bufs=1) as ktp,
            tc.tile_pool(name="v1p", bufs=1) as v1p,
            tc.tile_pool(name="expmtp", bufs=1) as expmtp,
            tc.tile_pool(name="qtp", bufs=2) as qtp,
            tc.tile_pool(name="ptp", bufs=2 * NKV) as ptp,
            tc.tile_pool(name="stage", bufs=3) as stagep,
            tc.tile_pool(name="outp", bufs=4) as outp,
            tc.tile_pool(name="smallp", bufs=4) as smallp,
            tc.tile_pool(name="stp", bufs=2, space="PSUM") as stp,
            tc.tile_pool(name="avp", bufs=2, space="PSUM") as avp,
            tc.tile_pool(name="trp", bufs=2, space="PSUM") as trp,
        ):
            ident = constp.tile([128, 128], F16)
            make_identity(nc, ident)
            bias_t = constp.tile([128, 1], F32)
            nc.any.memset(bias_t[:], EXP_BIAS)

            def one_pass():
                # ---- K^T / V1: direct fp16 DMA (host pre-laid-out)
                KT = ktp.tile([128, kvlen], F16, name="KT")
                nc.sync.dma_start(KT[:], kt_dram[:, :])
                V1 = v1p.tile([128, NKV, d + 1], F16, name="V1")
                nc.sync.dma_start(V1[:], v1_dram[:, :, :])

                # ---- expMT: [128 kv-part, c, q] = exp(mask^T), PE-transposed
                expMT = expmtp.tile([128, NKV, qlen], F16, name="expmt")
                for t in range(NQT):
                    mstage = stagep.tile([128, kvlen], F16, tag="stage",
                                         name="mstage")
                    nc.sync.dma_start(mstage[:], m_dram[t * 128:(t + 1) * 128, :])
                    for c0 in range(0, NKV, 4):
                        trt = trp.tile([128, 512], F16, tag="tr", name="trm")
                        for j in range(4):
                            c = c0 + j
                            nc.tensor.transpose(
                                trt[:, j * 128:(j + 1) * 128],
                                mstage[:, c * 128:(c + 1) * 128], ident,
                            )
                        # one ACT exp over 4 chunks, strided into expMT
                        nc.scalar.activation(
                            expMT[:, c0:c0 + 4, t * 128:(t + 1) * 128],
                            trt[:].rearrange("p (j q) -> p j q", j=4),
                            EXPF,
                        )

                # ---- main loop: software-pipelined over (head, q-half)
                # stages.  AV matmuls of stage N-1 are emitted after the S
                # matmuls of stage N, so the ACT exps of stage N overlap
                # the PE's AV work of stage N-1 (instead of idling).
                def emit_qdma(h):
                    qstage = stagep.tile([128, NQT, d], F16, tag="stage",
                                         name="qstage")
                    nc.sync.dma_start(
                        qstage[:], q_dram[h].rearrange("(t p) d -> p t d", p=128)
                    )
                    return qstage

                def emit_qtrans(qstage):
                    QT = qtp.tile([128, qlen], F16, name="QT")  # [d part, q]
                    for t0 in range(0, NQT, 4):
                        trt = trp.tile([128, 512], F16, tag="tr", name="trq")
                        for j in range(4):
                            nc.tensor.transpose(
                                trt[:, j * 128:(j + 1) * 128],
                                qstage[:, t0 + j, :], ident,
                            )
                        nc.vector.tensor_copy(
                            QT[:, t0 * 128:(t0 + 4) * 128], trt[:]
                        )
                    return QT

                def emit_av_qs(pts, h, qh, qs):
                    av = avp.tile([128, d + 1], F32, tag="av", name="av")
                    for c in range(NKV):
                        nc.tensor.matmul(
                            av[:],
                            lhsT=pts[c][:, qs * 128:(qs + 1) * 128],
                            rhs=V1[:, c, :],
                            start=(c == 0),
                            stop=(c == NKV - 1),
                        )
                    rec = smallp.tile([128, 1], F32, tag="rec", name="rec")
                    nc.vector.reciprocal(rec[:], av[:, d:d + 1])
                    ot = outp.tile([128, d], F16, tag="out", name="ot")
                    nc.vector.tensor_scalar_mul(ot[:], av[:, 0:d], rec[:])
                    q0 = (qh * QSUB + qs) * 128
                    nc.sync.dma_start(o_dram[h, q0:q0 + 128, :], ot[:])

                def emit_stage(QT, h, qh, prev):
                    """S/exp/mul for (h, qh), with the previous stage's AV
                    accumulation chains interleaved between S chunks so the
                    PE fills its exp-drain stalls with AV work."""
                    q_off = qh * QHS
                    pts = [
                        ptp.tile([128, QHS], F16, tag="pt",
                                 name=f"pt{h}_{qh}_{c}")
                        for c in range(NKV)
                    ]
                    for c in range(NKV):
                        st = stp.tile([128, QHS], F32, tag="st", name="st")
                        for s0 in range(0, QHS, 512):
                            nc.tensor.matmul(
                                st[:, s0:s0 + 512],
                                lhsT=KT[:, c * 128:(c + 1) * 128],
                                rhs=QT[:, q_off + s0:q_off + s0 + 512],
                                start=True,
                                stop=True,
                            )
                        nc.scalar.activation(
                            pts[c][:], st[:], EXPF, bias=bias_t[:], scale=SCALE,
                        )
                        nc.vector.tensor_mul(
                            pts[c][:], pts[c][:], expMT[:, c, q_off:q_off + QHS],
                        )
                        if prev is not None and c % 2 == 1:
                            emit_av_qs(*prev, qs=c // 2)
                    return pts

                qstage = emit_qdma(0)
                QT = emit_qtrans(qstage)
                prev = None  # (pts, h, qh) awaiting AV
                for h in range(n_heads):
                    if h + 1 < n_heads:
                        next_qstage = emit_qdma(h + 1)
                    for qh in range(NQH):
                        pts = emit_stage(QT, h, qh, prev)
                        prev = (pts, h, qh)
                    if h + 1 < n_heads:
                        QT = emit_qtrans(next_qstage)
                for qs in range(QSUB):
                    emit_av_qs(*prev, qs=qs)

            if hw_loop and repeat > 1:
                with tc.For_i(0, repeat, 1, name="rep"):
                    one_pass()
            else:
                for _ in range(repeat):
                    one_pass()

    nc.compile()
    return nc


def host_prep(query_layer, key_layer, value_layer, attention_mask):
    """Full fp32 inputs -> concatenated per-core fp16 arrays (axis 0 = core)."""
    q = np.asarray(query_layer)
    k = np.asarray(key_layer)
    v = np.asarray(value_layer)
    m = np.asarray(attention_mask)

    # q: (2, 32, 2048, 128) -> (64, 2048, 128) fp16; rows 8i..8i+8 = core i
    q16 = q.astype(np.float16).reshape(B * H, QLEN, D)

    NKV = KVLEN // 128
    kt_list, v1_list, m_list = [], [], []
    for i in range(N_CORES):
        b = i // (N_CORES // B)
        h0 = (i % (N_CORES // B)) * HEADS_PER_CORE
        g = h0 // (H // G)
        kt_list.append(np.ascontiguousarray(k[b, g].T.astype(np.float16)))
        vv = v[b, g].astype(np.float16).reshape(NKV, 128, D).transpose(1, 0, 2)
        v1 = np.empty((128, NKV, D + 1), np.float16)
        v1[:, :, :D] = vv
        v1[:, :, D] = 1.0
        v1_list.append(v1)
        m_list.append(m[b, 0])
    kt_g = np.concatenate(kt_list, axis=0)
    v1_g = np.concatenate(v1_list, axis=0)
    m_g = np.concatenate(m_list, axis=0).astype(np.float16)
    return {"q": q16, "kt": kt_g, "v1": v1_g, "mask": m_g}


def assemble_output(out_g):
    """(64*2048, 128) fp16 core-concat -> (2048, 2, 4096) fp32 context."""
    o = out_g.reshape(B, H, QLEN, D)
    return np.ascontiguousarray(
        o.transpose(2, 0, 1, 3).reshape(QLEN, B, H * D), dtype=np.float32
    )


def ref_numpy(q, k, v, m):
    """fp32 numpy reference for correctness checks (per-core semantics)."""
    b, h, qn, d = q.shape
    g = k.shape[1]
    r = h // g
    qg = q.reshape(b, g, r, qn, d)
    s = np.einsum("bgrqd,bgkd->bgrqk", qg, k) / (math.sqrt(d) * 2.0) * 2.0
    s = s + m[:, None]
    s = s - s.max(axis=-1, keepdims=True)
    p = np.exp(s)
    p /= p.sum(axis=-1, keepdims=True)
    o = np.einsum("bgrqk,bgkd->bgrqd", p, v).reshape(b, h, qn, d)
    return o.transpose(2, 0, 1, 3).reshape(qn, b, h * d)


# ---------------------------------------------------------------------------
# Host-side entry: cached PJRT dispatch across the 8 axon NeuronCores.
#
# run_bass_kernel_spmd builds a fresh jax.jit per call (retrace + XLA
# compile + NEFF reload every time); since kernel() may be called
# repeatedly, we build the same shard_map executable once (identical to
# what run_bass_kernel_spmd does under axon, via the same _bass_exec_p
# primitive) and cache it.
# ---------------------------------------------------------------------------

import jax
from jax.experimental.shard_map import shard_map
from jax.sharding import Mesh, PartitionSpec

from concourse.bass2jax import (
    _bass_exec_p,
    install_neuronx_cc_hook,
    partition_id_tensor,
)

_EXEC_CACHE = {}


def _make_cached_fn(nc, n_cores=N_CORES):
    install_neuronx_cc_hook()
    partition_name = nc.partition_id_tensor.name if nc.partition_id_tensor else None
    in_names, out_names, out_avals, out_shapes = [], [], [], []
    for alloc in nc.m.functions[0].allocations:
        if not isinstance(alloc, mybir.MemoryLocationSet):
            continue
        name = alloc.memorylocations[0].name
        if alloc.kind == "ExternalInput":
            if name != partition_name:
                in_names.append(name)
        elif alloc.kind == "ExternalOutput":
            out_names.append(name)
            shape = tuple(alloc.tensor_shape)
            dtype = mybir.dt.np(alloc.dtype)
            out_avals.append(jax.core.ShapedArray(shape, dtype))
            out_shapes.append((shape, dtype))
    n_params = len(in_names)
    all_in_names = list(in_names) + list(out_names)
    if partition_name is not None:
        all_in_names.append(partition_name)
    donate = tuple(range(n_params, n_params + len(out_names)))

    def _body(*args):
        operands = list(args)
        if partition_name is not None:
            operands.append(partition_id_tensor())
        outs = _bass_exec_p.bind(
            *operands,
            out_avals=tuple(out_avals),
            in_names=tuple(all_in_names),
            out_names=tuple(out_names),
            lowering_input_output_aliases=(),
            sim_require_finite=True,
            sim_require_nnan=True,
            nc=nc,
        )
        return tuple(outs)

    devices = jax.devices()[:n_cores]
    mesh = Mesh(np.asarray(devices), ("core",))
    n_io = n_params + len(out_names)
    sharded = jax.jit(
        shard_map(
            _body,
            mesh=mesh,
            in_specs=(PartitionSpec("core"),) * n_io,
            out_specs=(PartitionSpec("core"),) * len(out_names),
            check_rep=False,
        ),
        donate_argnums=donate,
        keep_unused=True,
    )
    return sharded, in_names, out_names, out_shapes


def get_exec(repeat=1, hw_loop=False):
    key = (repeat, hw_loop)
    if key not in _EXEC_CACHE:
        nc = build_program(repeat=repeat, hw_loop=hw_loop)
        _EXEC_CACHE[key] = _make_cached_fn(nc)
    return _EXEC_CACHE[key]


def run_exec(ex, in_map_global, n_cores=N_CORES):
    """in_map_global: {name: concat-over-cores array (axis 0)}."""
    sharded, in_names, out_names, out_shapes = ex
    concat_zeros = [
        np.zeros((n_cores * s[0], *s[1:]), dt) for s, dt in out_shapes
    ]
    outs = sharded(*[in_map_global[n] for n in in_names], *concat_zeros)
    jax.block_until_ready(outs)
    return {n: np.asarray(o) for n, o in zip(out_names, outs)}


def kernel(query_layer, key_layer, value_layer, attention_mask):
    """Full-input entry point: shards across 8 NeuronCores, returns the
    full (2048, 2, 4096) fp32 context."""
    ins = host_prep(query_layer, key_layer, value_layer, attention_mask)
    ex = get_exec()
    out = run_exec(ex, ins)["out"]
    return assemble_output(out)
